# revision 10
# baseline (speedup 1.0000x reference)
"""MultiHeadAttention Trainium2 Bass kernel, 8-core (batch x head-group) sharded.

Reference computation (B=4, S=2048, D=1024, H=16, d_k=64):
    Q = query @ W_q.T ; K = key @ W_k.T ; V = value @ W_v.T
    per head: attn = softmax(Q K^T / 8) @ V
    out = concat_heads(attn) @ W_o.T

Sharding: core c handles batch b = c // 2 and head-group hg = c % 2 (8 heads,
a 512-wide slice of the model dim). The host pre-transposes and pre-tiles
activations/weights into DMA-friendly layouts (contiguous per SBUF partition);
core-pair partial outputs (row-parallel W_o) are summed on the host while
unsharding.

Per-core dataflow (all matmul inputs float32r; contraction always on the
partition axis):
    K.T[d', s] = (W_k.T slice).T @ x_k.T    (d' on partitions)
    Q.T[d', s] likewise, projected per 512-wide q-block
    V[s, d']   = (x_v.T).T @ W_v.T          (natural layout, + ones column)
    S.T[k, q]  = (K_h.T).T @ Q_h.T          (two heads row-packed, K=64,
                                             both written to one 2-bank tile)
    expS.T     = exp(S.T / 8)               (one ACT op per k-tile, 1024 free)
    O.T+denom  = [V_h | 1].T @ expS.T       (M=65, accumulated over 16 k tiles)
    O.T norm   = O.T * (1/denom)            (copy out of PSUM, then DVE
                                             reciprocal + gpsimd broadcast)
    out[s, :]  = O.T.T @ W_o.T slice        (partial; host adds core pairs)
"""
import sys

sys.path.insert(0, "/opt/trn_rl_repo")

import numpy as np

import concourse.bass as bass  # noqa: F401
import concourse.tile as tile
from concourse import bacc, mybir
from concourse.bass_utils import run_bass_kernel_spmd

F32R = mybir.dt.float32r
F32 = mybir.dt.float32
EXP = mybir.ActivationFunctionType.Exp
MULT = mybir.AluOpType.mult

B, S, D = 4, 2048, 1024
H_PER_CORE = 8      # heads per core
DH = 64             # head dim
DP = 512            # per-core model-dim slice (8 heads x 64)
NT = 4              # d' tiles / head pairs per core
SB = 4              # 512-wide s/q blocks
KT = 16             # 128-wide k tiles
PKT = 8             # 128-wide contraction tiles for projections (D / 128)
VW = DH + 1         # V columns per head incl. ones column

_RUN_KWARGS = {}
_LAST_RESULT = []


def build_nc():
    nc = bacc.Bacc("TRN2", target_bir_lowering=False, debug=False)

    # activations pre-tiled on host: [sb, p, kt, 512], contiguous per partition
    xqt = nc.dram_tensor("xqt", [SB, 128, PKT, 512], F32R, kind="ExternalInput")
    xkt = nc.dram_tensor("xkt", [SB, 128, PKT, 512], F32R, kind="ExternalInput")
    xvt = nc.dram_tensor("xvt", [SB, 128, PKT, 512], F32R, kind="ExternalInput")
    # weights pre-tiled: [p, kt, m]
    wqt = nc.dram_tensor("wqt", [128, PKT, DP], F32R, kind="ExternalInput")
    wkt = nc.dram_tensor("wkt", [128, PKT, DP], F32R, kind="ExternalInput")
    wvt = nc.dram_tensor("wvt", [128, PKT, DP], F32R, kind="ExternalInput")
    wot = nc.dram_tensor("wot", [128, NT, D], F32R, kind="ExternalInput")
    out = nc.dram_tensor("out", [S, D], F32, kind="ExternalOutput")

    with tile.TileContext(nc) as tc:
        with tc.tile_pool(name="persist", bufs=1) as persist, \
             tc.tile_pool(name="psum", bufs=3, space="PSUM") as psum:

            # ---- persistent SBUF ----
            wq_s = persist.tile([128, PKT, DP], F32R)
            wot_s = persist.tile([128, NT, D], F32R)
            kt_s = persist.tile([128, NT, S], F32R)          # K.T
            vext_s = persist.tile([128, KT, H_PER_CORE * VW], F32R)  # [V_h | 1]

            nc.gpsimd.dma_start(wq_s[:], wqt[:])
            nc.gpsimd.dma_start(wot_s[:], wot[:])
            # ones columns for the denominator rows (V part is written below)
            ones_f = persist.tile([128, KT, H_PER_CORE], F32)
            nc.vector.memset(ones_f[:], 1.0)
            nc.vector.tensor_copy(
                vext_s[:].rearrange("p k (h c) -> p k h c", c=VW)[:, :, :, DH:DH + 1],
                ones_f[:, :, :, None],
            )

            # ============ phase 1+2: V and K.T projections ============
            with tc.tile_pool(name="proj", bufs=2) as proj:
                wk_s = proj.tile([128, PKT, DP], F32R, bufs=1)
                wv_s = proj.tile([128, PKT, DP], F32R, bufs=1)
                nc.gpsimd.dma_start(wv_s[:], wvt[:])
                nc.gpsimd.dma_start(wk_s[:], wkt[:])

                # V projection into [V_h | 1] layout
                for sg in range(SB):
                    xv_b = proj.tile([128, PKT, 512], F32R, tag="xv",
                                     name=f"xv_{sg}")
                    nc.sync.dma_start(xv_b[:], xvt[sg])
                    for half in range(4):
                        st = sg * 4 + half
                        ps = psum.tile([128, 512], F32, tag="sc",
                                       name=f"psv_{st}")
                        for kt in range(PKT):
                            nc.tensor.matmul(
                                ps[:], xv_b[:, kt, half * 128:(half + 1) * 128],
                                wv_s[:, kt, :],
                                start=kt == 0, stop=kt == PKT - 1,
                            )
                        nc.vector.tensor_copy(
                            vext_s[:, st, :].rearrange(
                                "p (h c) -> p h c", c=VW)[:, :, 0:DH],
                            ps[:].rearrange("p (h c) -> p h c", c=DH),
                        )

                # K.T projection
                for sb in range(SB):
                    xk_b = proj.tile([128, PKT, 512], F32R, tag="xk",
                                     name=f"xk_{sb}")
                    nc.sync.dma_start(xk_b[:], xkt[sb])
                    for t in range(NT):
                        ps = psum.tile([128, 512], F32, tag="sc",
                                       name=f"psk_{sb}_{t}")
                        for kt in range(PKT):
                            nc.tensor.matmul(
                                ps[:],
                                wk_s[:, kt, t * 128:(t + 1) * 128],
                                xk_b[:, kt, :],
                                start=kt == 0, stop=kt == PKT - 1,
                            )
                        nc.vector.tensor_copy(
                            kt_s[:, t, sb * 512:(sb + 1) * 512], ps[:])

            # ============ phase 3: attention + W_o, per 512-wide q block ==========
            with tc.tile_pool(name="att", bufs=2) as att:
                qt_tiles = {}

                def qt_proj(qb, t):
                    if t == 0:
                        xq_b = att.tile([128, PKT, 512], F32R, tag="xq", bufs=1,
                                        name=f"xq_{qb}")
                        nc.sync.dma_start(xq_b[:], xqt[qb])
                        qt_tiles[qb] = (att.tile([128, NT, 512], F32R, tag="qt",
                                                 name=f"qt_{qb}"), xq_b)
                    qt_b, xq_b = qt_tiles[qb]
                    ps = psum.tile([128, 512], F32, tag="sc", name=f"psq_{qb}_{t}")
                    for kt in range(PKT):
                        nc.tensor.matmul(
                            ps[:], wq_s[:, kt, t * 128:(t + 1) * 128],
                            xq_b[:, kt, :],
                            start=kt == 0, stop=kt == PKT - 1,
                        )
                    nc.vector.tensor_copy(qt_b[:, t, :], ps[:])

                ot_tiles = {}

                def wo_stage(qb, si):
                    ot_b = ot_tiles[qb]
                    st = qb * 4 + si
                    ssl = slice(si * 128, (si + 1) * 128)
                    for dm in range(2):
                        ps = psum.tile([128, 512], F32, tag="sc",
                                       name=f"pso_{st}_{dm}")
                        for t in range(NT):
                            nc.tensor.matmul(
                                ps[:], ot_b[:, t, ssl],
                                wot_s[:, t, dm * 512:(dm + 1) * 512],
                                start=t == 0, stop=t == NT - 1,
                            )
                        ob = att.tile([128, 512], F32, tag="ob", bufs=3,
                                      name=f"ob_{st}_{dm}")
                        nc.vector.tensor_copy(ob[:], ps[:])
                        nc.sync.dma_start(
                            out[st * 128:(st + 1) * 128,
                                dm * 512:(dm + 1) * 512],
                            ob[:])

                for t in range(NT):
                    qt_proj(0, t)

                for qb in range(SB):
                    qt_b, _ = qt_tiles[qb]
                    ot_b = att.tile([128, NT, 512], F32R, tag="ot",
                                    name=f"ot_{qb}")
                    ot_tiles[qb] = ot_b
                    for t in range(NT):
                        # two heads: A on partitions 0:64, B on 64:128
                        ota = psum.tile([65, 512], F32, tag="ot", bufs=2,
                                        name=f"ota_{qb}_{t}")
                        otb = psum.tile([65, 512], F32, tag="ot", bufs=2,
                                        name=f"otb_{qb}_{t}")
                        ha, hb = 2 * t, 2 * t + 1
                        for kt in range(KT):
                            ksl = slice(kt * 128, (kt + 1) * 128)
                            sc = psum.tile([128, 2, 512], F32, tag="sc",
                                           name=f"sc_{qb}_{t}_{kt}")
                            nc.tensor.matmul(
                                sc[:, 0, :], kt_s[0:64, t, ksl],
                                qt_b[0:64, t, :],
                                start=True, stop=True, tile_position=(0, 0),
                            )
                            nc.tensor.matmul(
                                sc[:, 1, :], kt_s[64:128, t, ksl],
                                qt_b[64:128, t, :],
                                start=True, stop=True, tile_position=(64, 0),
                            )
                            e = att.tile([128, 2, 512], F32R, tag="exp", bufs=6,
                                         name=f"e_{qb}_{t}_{kt}")
                            nc.scalar.activation(e[:], sc[:], EXP, scale=0.125)
                            nc.tensor.matmul(
                                ota[:], vext_s[:, kt, ha * VW:(ha + 1) * VW],
                                e[:, 0, :],
                                start=kt == 0, stop=kt == KT - 1,
                            )
                            nc.tensor.matmul(
                                otb[:], vext_s[:, kt, hb * VW:(hb + 1) * VW],
                                e[:, 1, :],
                                start=kt == 0, stop=kt == KT - 1,
                            )
                        # evacuate PSUM fast, normalize from SBUF afterwards
                        for nm, ot_ps, psl in (("a", ota, slice(0, 64)),
                                               ("b", otb, slice(64, 128))):
                            otr = att.tile([64, 512], F32, tag="otr", bufs=4,
                                           name=f"otr{nm}_{qb}_{t}")
                            nc.vector.tensor_copy(otr[:], ot_ps[0:64, :])
                            dn = att.tile([1, 512], F32, tag="dn", bufs=4,
                                          name=f"dn{nm}_{qb}_{t}")
                            nc.vector.tensor_copy(dn[:], ot_ps[64:65, :])
                            rd = att.tile([1, 512], F32, tag="rd", bufs=2,
                                          name=f"rd{nm}_{qb}_{t}")
                            nc.vector.reciprocal_approx_fast(rd[:], dn[:])
                            rb = att.tile([64, 512], F32, tag="rb", bufs=2,
                                          name=f"rb{nm}_{qb}_{t}")
                            nc.gpsimd.partition_broadcast(rb[:], rd[:])
                            nc.vector.tensor_tensor(
                                ot_b[psl, t, :], otr[0:64, :], rb[:], MULT)
                        # spread next q block's Q.T proj / previous block's W_o
                        if qb + 1 < SB:
                            qt_proj(qb + 1, t)
                        if qb > 0:
                            wo_stage(qb - 1, t)

                for si in range(4):
                    wo_stage(SB - 1, si)
    nc.compile()
    return nc


_NC_CACHE = []


def _tile_x(x):
    # x: [S, D] -> x.T tiled [SB, 128, PKT, 512] with
    # tiled[sb, p, kt, s] = x.T[kt*128 + p, sb*512 + s]
    return np.ascontiguousarray(
        x.T.reshape(PKT, 128, SB, 512).transpose(2, 1, 0, 3))


def _tile_w(wt, nt, m):
    # wt: [D_in, m] (already transposed weight slice) -> [128, nt, m]
    return np.ascontiguousarray(wt.reshape(nt, 128, m).transpose(1, 0, 2))


def kernel(**inputs):
    query = np.asarray(inputs["query"], dtype=np.float32)
    key = np.asarray(inputs["key"], dtype=np.float32)
    value = np.asarray(inputs["value"], dtype=np.float32)
    w_q = np.asarray(inputs["W_q"], dtype=np.float32)
    w_k = np.asarray(inputs["W_k"], dtype=np.float32)
    w_v = np.asarray(inputs["W_v"], dtype=np.float32)
    w_o = np.asarray(inputs["W_o"], dtype=np.float32)

    in_maps = []
    for c in range(8):
        b, hg = c // 2, c % 2
        dsl = slice(hg * DP, (hg + 1) * DP)
        in_maps.append({
            "xqt": _tile_x(query[b]),
            "xkt": _tile_x(key[b]),
            "xvt": _tile_x(value[b]),
            "wqt": _tile_w(w_q[dsl, :].T, PKT, DP),
            "wkt": _tile_w(w_k[dsl, :].T, PKT, DP),
            "wvt": _tile_w(w_v[dsl, :].T, PKT, DP),
            "wot": _tile_w(w_o[:, dsl].T, NT, D),
        })

    if not _NC_CACHE:
        _NC_CACHE.append(build_nc())
    nc = _NC_CACHE[0]
    res = run_bass_kernel_spmd(nc, in_maps, core_ids=list(range(8)),
                               **_RUN_KWARGS)
    _LAST_RESULT.clear()
    _LAST_RESULT.append(res)
    parts = [r["out"] for r in res.results]
    full = np.empty((B, S, D), dtype=np.float32)
    for b in range(B):
        full[b] = parts[2 * b] + parts[2 * b + 1]
    return full


# revision 14
# speedup vs baseline: 1.1670x; 1.1670x over previous
"""MultiHeadAttention Trainium2 Bass kernel, 8-core (batch x head-group) sharded.

Reference computation (B=4, S=2048, D=1024, H=16, d_k=64):
    Q = query @ W_q.T ; K = key @ W_k.T ; V = value @ W_v.T
    per head: attn = softmax(Q K^T / 8) @ V
    out = concat_heads(attn) @ W_o.T

Sharding: core c handles batch b = c // 2 and head-group hg = c % 2 (8 heads,
a 512-wide slice of the model dim). The host pre-transposes and pre-tiles
activations/weights into DMA-friendly layouts (contiguous per SBUF partition);
core-pair partial outputs (row-parallel W_o) are summed on the host while
unsharding.

Per-core dataflow (all matmul inputs float32r; contraction always on the
partition axis):
    K.T[d', s] = (W_k.T slice).T @ x_k.T    (d' on partitions)
    Q.T[d', s] likewise, projected per 512-wide q-block
    V[s, d']   = (x_v.T).T @ W_v.T          (natural layout, + ones column)
    S.T[k, q]  = (K_h.T).T @ Q_h.T          (two heads row-packed, K=64,
                                             both written to one 2-bank tile)
    expS.T     = exp(S.T / 8)               (one ACT op per k-tile, 1024 free)
    O.T+denom  = [V_h | 1].T @ expS.T       (M=65, accumulated over 16 k tiles)
    O.T norm   = O.T * (1/denom)            (copy out of PSUM, then DVE
                                             reciprocal + gpsimd broadcast)
    out[s, :]  = O.T.T @ W_o.T slice        (partial; host adds core pairs)

Scheduling notes: scores->exp->PV runs as a depth-3 pipeline through six PSUM
banks (tag "sc") plus two O.T accumulator banks (tag "ot"); W_o and next-block
Q.T projections are emitted as small chunks inside the k-loop so the in-order
PE stream never starves the scalar engine at block boundaries.
"""
import sys

sys.path.insert(0, "/opt/trn_rl_repo")

import numpy as np

import concourse.bass as bass  # noqa: F401
import concourse.tile as tile
from concourse import bacc, mybir
from concourse.bass_utils import run_bass_kernel_spmd

F32R = mybir.dt.float32r
F32 = mybir.dt.float32
EXP = mybir.ActivationFunctionType.Exp
MULT = mybir.AluOpType.mult

B, S, D = 4, 2048, 1024
H_PER_CORE = 8      # heads per core
DH = 64             # head dim
DP = 512            # per-core model-dim slice (8 heads x 64)
NT = 4              # d' tiles / head pairs per core
SB = 4              # 512-wide s/q blocks
KT = 16             # 128-wide k tiles
PKT = 8             # 128-wide contraction tiles for projections (D / 128)
VW = DH + 1         # V columns per head incl. ones column

_RUN_KWARGS = {}
_LAST_RESULT = []


def build_nc():
    nc = bacc.Bacc("TRN2", target_bir_lowering=False, debug=False)

    # activations pre-tiled on host: [sb, p, kt, 512], contiguous per partition
    xqt = nc.dram_tensor("xqt", [SB, 128, PKT, 512], F32R, kind="ExternalInput")
    xkt = nc.dram_tensor("xkt", [SB, 128, PKT, 512], F32R, kind="ExternalInput")
    xvt = nc.dram_tensor("xvt", [SB, 128, PKT, 512], F32R, kind="ExternalInput")
    # weights pre-tiled: [p, kt, m]
    wqt = nc.dram_tensor("wqt", [128, PKT, DP], F32R, kind="ExternalInput")
    wkt = nc.dram_tensor("wkt", [128, PKT, DP], F32R, kind="ExternalInput")
    wvt = nc.dram_tensor("wvt", [128, PKT, DP], F32R, kind="ExternalInput")
    wot = nc.dram_tensor("wot", [128, NT, D], F32R, kind="ExternalInput")
    out = nc.dram_tensor("out", [S, D], F32, kind="ExternalOutput")

    with tile.TileContext(nc) as tc:
        with tc.tile_pool(name="persist", bufs=1) as persist, \
             tc.tile_pool(name="psum", bufs=3, space="PSUM") as psum:

            # ---- persistent SBUF ----
            wq_s = persist.tile([128, PKT, DP], F32R)
            wot_s = persist.tile([128, NT, D], F32R)
            kt_s = persist.tile([128, NT, S], F32R)          # K.T
            vext_s = persist.tile([128, KT, H_PER_CORE * VW], F32R)  # [V_h | 1]

            # ones columns for the denominator rows (V part is written below)
            ones_f = persist.tile([128, KT, H_PER_CORE], F32)
            nc.vector.memset(ones_f[:], 1.0)
            nc.vector.tensor_copy(
                vext_s[:].rearrange("p k (h c) -> p k h c", c=VW)[:, :, :, DH:DH + 1],
                ones_f[:, :, :, None],
            )

            # ============ phase 1+2: V and K.T projections ============
            with tc.tile_pool(name="proj", bufs=2) as proj:
                wk_s = proj.tile([128, PKT, DP], F32R, bufs=1)
                wv_s = proj.tile([128, PKT, DP], F32R, bufs=1)
                # gpsimd (SWDGE) queue: weights in order of first use
                nc.gpsimd.dma_start(wv_s[:], wvt[:])
                nc.gpsimd.dma_start(wk_s[:], wkt[:])
                nc.gpsimd.dma_start(wq_s[:], wqt[:])
                nc.gpsimd.dma_start(wot_s[:], wot[:])

                # V projection into [V_h | 1] layout
                for sg in range(SB):
                    xv_b = proj.tile([128, PKT, 512], F32R, tag="xv",
                                     name=f"xv_{sg}")
                    nc.sync.dma_start(xv_b[:], xvt[sg])
                    for half in range(4):
                        st = sg * 4 + half
                        ps = psum.tile([128, 512], F32, tag="sc",
                                       name=f"psv_{st}")
                        for kt in range(PKT):
                            nc.tensor.matmul(
                                ps[:], xv_b[:, kt, half * 128:(half + 1) * 128],
                                wv_s[:, kt, :],
                                start=kt == 0, stop=kt == PKT - 1,
                            )
                        nc.vector.tensor_copy(
                            vext_s[:, st, :].rearrange(
                                "p (h c) -> p h c", c=VW)[:, :, 0:DH],
                            ps[:].rearrange("p (h c) -> p h c", c=DH),
                        )

                # K.T projection
                for sb in range(SB):
                    xk_b = proj.tile([128, PKT, 512], F32R, tag="xk",
                                     name=f"xk_{sb}")
                    nc.sync.dma_start(xk_b[:], xkt[sb])
                    for t in range(NT):
                        ps = psum.tile([128, 512], F32, tag="sc",
                                       name=f"psk_{sb}_{t}")
                        for kt in range(PKT):
                            nc.tensor.matmul(
                                ps[:],
                                wk_s[:, kt, t * 128:(t + 1) * 128],
                                xk_b[:, kt, :],
                                start=kt == 0, stop=kt == PKT - 1,
                            )
                        nc.vector.tensor_copy(
                            kt_s[:, t, sb * 512:(sb + 1) * 512], ps[:])

            # ============ phase 3: attention + W_o, per 512-wide q block ==========
            with tc.tile_pool(name="att", bufs=2) as att:
                qt_tiles = {}

                def xq_load(qb):
                    xq_b = att.tile([128, PKT, 512], F32R, tag="xq", bufs=1,
                                    name=f"xq_{qb}")
                    nc.sync.dma_start(xq_b[:], xqt[qb])
                    qt_tiles[qb] = (att.tile([128, NT, 512], F32R, tag="qt",
                                             name=f"qt_{qb}"), xq_b)

                def qt_proj(qb, t):
                    qt_b, xq_b = qt_tiles[qb]
                    ps = psum.tile([128, 512], F32, tag="sc", name=f"psq_{qb}_{t}")
                    for kt in range(PKT):
                        nc.tensor.matmul(
                            ps[:], wq_s[:, kt, t * 128:(t + 1) * 128],
                            xq_b[:, kt, :],
                            start=kt == 0, stop=kt == PKT - 1,
                        )
                    nc.vector.tensor_copy(qt_b[:, t, :], ps[:])

                ot_tiles = {}

                def wo_stage(qb, si, dm):
                    ot_b = ot_tiles[qb]
                    st = qb * 4 + si
                    ssl = slice(si * 128, (si + 1) * 128)
                    ps = psum.tile([128, 512], F32, tag="sc",
                                   name=f"pso_{st}_{dm}")
                    for t in range(NT):
                        nc.tensor.matmul(
                            ps[:], ot_b[:, t, ssl],
                            wot_s[:, t, dm * 512:(dm + 1) * 512],
                            start=t == 0, stop=t == NT - 1,
                        )
                    ob = att.tile([128, 512], F32, tag="ob", bufs=3,
                                  name=f"ob_{st}_{dm}")
                    nc.vector.tensor_copy(ob[:], ps[:])
                    nc.sync.dma_start(
                        out[st * 128:(st + 1) * 128, dm * 512:(dm + 1) * 512],
                        ob[:])

                from collections import deque
                from functools import partial

                pending = deque()
                FILL_SLOTS = (4, 9, 14)

                xq_load(0)
                qt_proj(0, 0)
                for tt in range(1, NT):
                    pending.append(partial(qt_proj, 0, tt))
                pending.append(partial(xq_load, 1))
                for tt in range(NT):
                    pending.append(partial(qt_proj, 1, tt))

                for qb in range(SB):
                    if qb >= 1:
                        # interleave W_o of the previous block with Q.T
                        # projection of the next one
                        nxt = []
                        if qb + 1 < SB:
                            nxt.append(partial(xq_load, qb + 1))
                            nxt.extend(partial(qt_proj, qb + 1, tt)
                                       for tt in range(NT))
                        wos = [partial(wo_stage, qb - 1, si, dm)
                               for si in range(4) for dm in range(2)]
                        merged = []
                        while nxt or wos:
                            if wos:
                                merged.append(wos.pop(0))
                            if nxt:
                                merged.append(nxt.pop(0))
                        pending.extend(merged)

                    qt_b, _ = qt_tiles[qb]
                    ot_b = att.tile([128, NT, 512], F32R, tag="ot",
                                    name=f"ot_{qb}")
                    ot_tiles[qb] = ot_b
                    for t in range(NT):

                        # two heads: A on partitions 0:64, B on 64:128
                        ota = psum.tile([65, 512], F32, tag="ot", bufs=2,
                                        name=f"ota_{qb}_{t}")
                        otb = psum.tile([65, 512], F32, tag="ot", bufs=2,
                                        name=f"otb_{qb}_{t}")
                        ha, hb = 2 * t, 2 * t + 1
                        for kt in range(KT):
                            ksl = slice(kt * 128, (kt + 1) * 128)
                            sc = psum.tile([128, 2, 512], F32, tag="sc",
                                           name=f"sc_{qb}_{t}_{kt}")
                            nc.tensor.matmul(
                                sc[:, 0, :], kt_s[0:64, t, ksl],
                                qt_b[0:64, t, :],
                                start=True, stop=True, tile_position=(0, 0),
                            )
                            nc.tensor.matmul(
                                sc[:, 1, :], kt_s[64:128, t, ksl],
                                qt_b[64:128, t, :],
                                start=True, stop=True, tile_position=(64, 0),
                            )
                            e = att.tile([128, 2, 512], F32R, tag="exp", bufs=6,
                                         name=f"e_{qb}_{t}_{kt}")
                            nc.scalar.activation(e[:], sc[:], EXP, scale=0.125)
                            nc.tensor.matmul(
                                ota[:], vext_s[:, kt, ha * VW:(ha + 1) * VW],
                                e[:, 0, :],
                                start=kt == 0, stop=kt == KT - 1,
                            )
                            nc.tensor.matmul(
                                otb[:], vext_s[:, kt, hb * VW:(hb + 1) * VW],
                                e[:, 1, :],
                                start=kt == 0, stop=kt == KT - 1,
                            )
                            if kt in FILL_SLOTS and pending:
                                pending.popleft()()
                        # evacuate PSUM fast, normalize from SBUF afterwards
                        for nm, ot_ps, psl in (("a", ota, slice(0, 64)),
                                               ("b", otb, slice(64, 128))):
                            otr = att.tile([64, 512], F32, tag="otr", bufs=4,
                                           name=f"otr{nm}_{qb}_{t}")
                            nc.vector.tensor_copy(otr[:], ot_ps[0:64, :])
                            dn = att.tile([1, 512], F32, tag="dn", bufs=4,
                                          name=f"dn{nm}_{qb}_{t}")
                            nc.vector.tensor_copy(dn[:], ot_ps[64:65, :])
                            rd = att.tile([1, 512], F32, tag="rd", bufs=2,
                                          name=f"rd{nm}_{qb}_{t}")
                            nc.vector.reciprocal_approx_fast(rd[:], dn[:])
                            rb = att.tile([64, 512], F32, tag="rb", bufs=2,
                                          name=f"rb{nm}_{qb}_{t}")
                            nc.gpsimd.partition_broadcast(rb[:], rd[:])
                            nc.vector.tensor_tensor(
                                ot_b[psl, t, :], otr[:], rb[:], MULT)

                while pending:
                    pending.popleft()()
                for si in range(4):
                    wo_stage(SB - 1, si, 0)
                    wo_stage(SB - 1, si, 1)
    nc.compile()
    return nc


_NC_CACHE = []


def _tile_x(x):
    # x: [S, D] -> x.T tiled [SB, 128, PKT, 512] with
    # tiled[sb, p, kt, s] = x.T[kt*128 + p, sb*512 + s]
    return np.ascontiguousarray(
        x.T.reshape(PKT, 128, SB, 512).transpose(2, 1, 0, 3))


def _tile_w(wt, nt, m):
    # wt: [D_in, m] (already transposed weight slice) -> [128, nt, m]
    return np.ascontiguousarray(wt.reshape(nt, 128, m).transpose(1, 0, 2))


def kernel(**inputs):
    query = np.asarray(inputs["query"], dtype=np.float32)
    key = np.asarray(inputs["key"], dtype=np.float32)
    value = np.asarray(inputs["value"], dtype=np.float32)
    w_q = np.asarray(inputs["W_q"], dtype=np.float32)
    w_k = np.asarray(inputs["W_k"], dtype=np.float32)
    w_v = np.asarray(inputs["W_v"], dtype=np.float32)
    w_o = np.asarray(inputs["W_o"], dtype=np.float32)

    in_maps = []
    for c in range(8):
        b, hg = c // 2, c % 2
        dsl = slice(hg * DP, (hg + 1) * DP)
        in_maps.append({
            "xqt": _tile_x(query[b]),
            "xkt": _tile_x(key[b]),
            "xvt": _tile_x(value[b]),
            "wqt": _tile_w(w_q[dsl, :].T, PKT, DP),
            "wkt": _tile_w(w_k[dsl, :].T, PKT, DP),
            "wvt": _tile_w(w_v[dsl, :].T, PKT, DP),
            "wot": _tile_w(w_o[:, dsl].T, NT, D),
        })

    if not _NC_CACHE:
        _NC_CACHE.append(build_nc())
    nc = _NC_CACHE[0]
    res = run_bass_kernel_spmd(nc, in_maps, core_ids=list(range(8)),
                               **_RUN_KWARGS)
    _LAST_RESULT.clear()
    _LAST_RESULT.append(res)
    parts = [r["out"] for r in res.results]
    full = np.empty((B, S, D), dtype=np.float32)
    for b in range(B):
        full[b] = parts[2 * b] + parts[2 * b + 1]
    return full


# revision 15
# speedup vs baseline: 1.2001x; 1.0284x over previous
"""MultiHeadAttention Trainium2 Bass kernel, 8-core (batch x head-group) sharded.

Reference computation (B=4, S=2048, D=1024, H=16, d_k=64):
    Q = query @ W_q.T ; K = key @ W_k.T ; V = value @ W_v.T
    per head: attn = softmax(Q K^T / 8) @ V
    out = concat_heads(attn) @ W_o.T

Sharding: core c handles batch b = c // 2 and head-group hg = c % 2 (8 heads,
a 512-wide slice of the model dim). The host pre-transposes and pre-tiles
activations/weights into DMA-friendly layouts (contiguous per SBUF partition);
core-pair partial outputs (row-parallel W_o) are summed on the host while
unsharding.

Per-core dataflow (all matmul inputs float32r; contraction always on the
partition axis):
    K.T[d', s] = (W_k.T slice).T @ x_k.T    (d' on partitions)
    Q.T[d', s] likewise, projected per 512-wide q-block
    V[s, d']   = (x_v.T).T @ W_v.T          (natural layout, + ones column)
    S.T[k, q]  = (K_h.T).T @ Q_h.T          (two heads row-packed, K=64,
                                             both written to one 2-bank tile)
    expS.T     = exp(S.T / 8)               (one ACT op per k-tile, 1024 free)
    O.T+denom  = [V_h | 1].T @ expS.T       (M=65, accumulated over 16 k tiles)
    O.T norm   = O.T * (1/denom)            (copy out of PSUM, then DVE
                                             reciprocal + gpsimd broadcast)
    out[s, :]  = O.T.T @ W_o.T slice        (partial; host adds core pairs)

Scheduling notes: scores->exp->PV runs as a depth-3 pipeline through six PSUM
banks (tag "sc") plus two O.T accumulator banks (tag "ot"); W_o and next-block
Q.T projections are emitted as small chunks inside the k-loop so the in-order
PE stream never starves the scalar engine at block boundaries.
"""
import sys

sys.path.insert(0, "/opt/trn_rl_repo")

import numpy as np

import concourse.bass as bass  # noqa: F401
import concourse.tile as tile
from concourse import bacc, mybir
from concourse.bass_utils import run_bass_kernel_spmd

F32R = mybir.dt.float32r
F32 = mybir.dt.float32
EXP = mybir.ActivationFunctionType.Exp
MULT = mybir.AluOpType.mult

B, S, D = 4, 2048, 1024
H_PER_CORE = 8      # heads per core
DH = 64             # head dim
DP = 512            # per-core model-dim slice (8 heads x 64)
NT = 4              # d' tiles / head pairs per core
SB = 4              # 512-wide s/q blocks
KT = 16             # 128-wide k tiles
PKT = 8             # 128-wide contraction tiles for projections (D / 128)
VW = DH + 1         # V columns per head incl. ones column

_RUN_KWARGS = {}
_LAST_RESULT = []


def build_nc():
    nc = bacc.Bacc("TRN2", target_bir_lowering=False, debug=False)

    # activations pre-tiled on host: [sb, p, kt, 512], contiguous per partition
    xqt = nc.dram_tensor("xqt", [SB, 128, PKT, 512], F32R, kind="ExternalInput")
    xkt = nc.dram_tensor("xkt", [SB, 128, PKT, 512], F32R, kind="ExternalInput")
    xvt = nc.dram_tensor("xvt", [SB, 128, PKT, 512], F32R, kind="ExternalInput")
    # weights pre-tiled: [p, kt, m]
    wqt = nc.dram_tensor("wqt", [128, PKT, DP], F32R, kind="ExternalInput")
    wkt = nc.dram_tensor("wkt", [128, PKT, DP], F32R, kind="ExternalInput")
    wvt = nc.dram_tensor("wvt", [128, PKT, DP], F32R, kind="ExternalInput")
    wot = nc.dram_tensor("wot", [128, NT, D], F32R, kind="ExternalInput")
    out = nc.dram_tensor("out", [S, D], F32, kind="ExternalOutput")

    with tile.TileContext(nc) as tc:
        with tc.tile_pool(name="persist", bufs=1) as persist, \
             tc.tile_pool(name="psum", bufs=3, space="PSUM") as psum:

            # ---- persistent SBUF ----
            wq_s = persist.tile([128, PKT, DP], F32R)
            wot_s = persist.tile([128, NT, D], F32R)
            kt_s = persist.tile([128, NT, S], F32R)          # K.T
            vext_s = persist.tile([128, KT, H_PER_CORE * VW], F32R)  # [V_h | 1]

            # ones columns for the denominator rows (V part is written below)
            ones_f = persist.tile([128, KT, H_PER_CORE], F32)
            nc.vector.memset(ones_f[:], 1.0)
            nc.vector.tensor_copy(
                vext_s[:].rearrange("p k (h c) -> p k h c", c=VW)[:, :, :, DH:DH + 1],
                ones_f[:, :, :, None],
            )

            # ============ phase 1+2: V and K.T projections ============
            with tc.tile_pool(name="proj", bufs=2) as proj:
                wk_s = proj.tile([128, PKT, DP], F32R, bufs=1)
                wv_s = proj.tile([128, PKT, DP], F32R, bufs=1)
                # gpsimd (SWDGE) queue: weights in order of first use
                nc.gpsimd.dma_start(wk_s[:], wkt[:])
                nc.gpsimd.dma_start(wq_s[:], wqt[:])
                nc.gpsimd.dma_start(wv_s[:], wvt[:])

                # K.T projection
                for sb in range(SB):
                    xk_b = proj.tile([128, PKT, 512], F32R, tag="xk",
                                     name=f"xk_{sb}")
                    nc.sync.dma_start(xk_b[:], xkt[sb])
                    for t in range(NT):
                        ps = psum.tile([128, 512], F32, tag="sc",
                                       name=f"psk_{sb}_{t}")
                        for kt in range(PKT):
                            nc.tensor.matmul(
                                ps[:],
                                wk_s[:, kt, t * 128:(t + 1) * 128],
                                xk_b[:, kt, :],
                                start=kt == 0, stop=kt == PKT - 1,
                            )
                        nc.vector.tensor_copy(
                            kt_s[:, t, sb * 512:(sb + 1) * 512], ps[:])

                # V projection into [V_h | 1] layout
                for sg in range(SB):
                    xv_b = proj.tile([128, PKT, 512], F32R, tag="xv",
                                     name=f"xv_{sg}")
                    nc.sync.dma_start(xv_b[:], xvt[sg])
                    for half in range(4):
                        st = sg * 4 + half
                        ps = psum.tile([128, 512], F32, tag="sc",
                                       name=f"psv_{st}")
                        for kt in range(PKT):
                            nc.tensor.matmul(
                                ps[:], xv_b[:, kt, half * 128:(half + 1) * 128],
                                wv_s[:, kt, :],
                                start=kt == 0, stop=kt == PKT - 1,
                            )
                        nc.vector.tensor_copy(
                            vext_s[:, st, :].rearrange(
                                "p (h c) -> p h c", c=VW)[:, :, 0:DH],
                            ps[:].rearrange("p (h c) -> p h c", c=DH),
                        )

            # ============ phase 3: attention + W_o, per 512-wide q block ==========
            with tc.tile_pool(name="att", bufs=2) as att:
                qt_tiles = {}

                def xq_load(qb):
                    xq_b = att.tile([128, PKT, 512], F32R, tag="xq", bufs=1,
                                    name=f"xq_{qb}")
                    nc.sync.dma_start(xq_b[:], xqt[qb])
                    qt_tiles[qb] = (att.tile([128, NT, 512], F32R, tag="qt",
                                             name=f"qt_{qb}"), xq_b)

                qt_ps = {}

                def qt_proj(qb, t, half=None):
                    qt_b, xq_b = qt_tiles[qb]
                    if half == 0 or half is None:
                        qt_ps[(qb, t)] = psum.tile([128, 512], F32, tag="sc",
                                                   name=f"psq_{qb}_{t}")
                    ps = qt_ps[(qb, t)]
                    kts = (range(PKT) if half is None else
                           range(half * PKT // 2, (half + 1) * PKT // 2))
                    for kt in kts:
                        nc.tensor.matmul(
                            ps[:], wq_s[:, kt, t * 128:(t + 1) * 128],
                            xq_b[:, kt, :],
                            start=kt == 0, stop=kt == PKT - 1,
                        )
                    if half == 1 or half is None:
                        nc.vector.tensor_copy(qt_b[:, t, :], ps[:])

                def wot_load():
                    nc.gpsimd.dma_start(wot_s[:], wot[:])

                ot_tiles = {}

                def wo_stage(qb, si, dm):
                    ot_b = ot_tiles[qb]
                    st = qb * 4 + si
                    ssl = slice(si * 128, (si + 1) * 128)
                    ps = psum.tile([128, 512], F32, tag="sc",
                                   name=f"pso_{st}_{dm}")
                    for t in range(NT):
                        nc.tensor.matmul(
                            ps[:], ot_b[:, t, ssl],
                            wot_s[:, t, dm * 512:(dm + 1) * 512],
                            start=t == 0, stop=t == NT - 1,
                        )
                    ob = att.tile([128, 512], F32, tag="ob", bufs=3,
                                  name=f"ob_{st}_{dm}")
                    nc.vector.tensor_copy(ob[:], ps[:])
                    nc.sync.dma_start(
                        out[st * 128:(st + 1) * 128, dm * 512:(dm + 1) * 512],
                        ob[:])

                from collections import deque
                from functools import partial

                pending = deque()
                FILL_SLOTS = (2, 5, 8, 11, 14)

                xq_load(0)
                qt_proj(0, 0)
                for tt in range(1, NT):
                    pending.append(partial(qt_proj, 0, tt, 0))
                    pending.append(partial(qt_proj, 0, tt, 1))
                pending.append(wot_load)
                pending.append(partial(xq_load, 1))
                for tt in range(NT):
                    pending.append(partial(qt_proj, 1, tt, 0))
                    pending.append(partial(qt_proj, 1, tt, 1))

                for qb in range(SB):
                    if qb >= 1:
                        # interleave W_o of the previous block with Q.T
                        # projection of the next one
                        nxt = []
                        if qb + 1 < SB:
                            nxt.append(partial(xq_load, qb + 1))
                            for tt in range(NT):
                                nxt.append(partial(qt_proj, qb + 1, tt, 0))
                                nxt.append(partial(qt_proj, qb + 1, tt, 1))
                        wos = [partial(wo_stage, qb - 1, si, dm)
                               for si in range(4) for dm in range(2)]
                        merged = []
                        while nxt or wos:
                            if wos:
                                merged.append(wos.pop(0))
                            if nxt:
                                merged.append(nxt.pop(0))
                        pending.extend(merged)

                    qt_b, _ = qt_tiles[qb]
                    ot_b = att.tile([128, NT, 512], F32R, tag="ot",
                                    name=f"ot_{qb}")
                    ot_tiles[qb] = ot_b
                    for t in range(NT):

                        # two heads: A on partitions 0:64, B on 64:128
                        ota = psum.tile([65, 512], F32, tag="ot", bufs=2,
                                        name=f"ota_{qb}_{t}")
                        otb = psum.tile([65, 512], F32, tag="ot", bufs=2,
                                        name=f"otb_{qb}_{t}")
                        ha, hb = 2 * t, 2 * t + 1
                        for kt in range(KT):
                            ksl = slice(kt * 128, (kt + 1) * 128)
                            sc = psum.tile([128, 2, 512], F32, tag="sc",
                                           name=f"sc_{qb}_{t}_{kt}")
                            nc.tensor.matmul(
                                sc[:, 0, :], kt_s[0:64, t, ksl],
                                qt_b[0:64, t, :],
                                start=True, stop=True, tile_position=(0, 0),
                            )
                            nc.tensor.matmul(
                                sc[:, 1, :], kt_s[64:128, t, ksl],
                                qt_b[64:128, t, :],
                                start=True, stop=True, tile_position=(64, 0),
                            )
                            e = att.tile([128, 2, 512], F32R, tag="exp", bufs=6,
                                         name=f"e_{qb}_{t}_{kt}")
                            nc.scalar.activation(e[:], sc[:], EXP, scale=0.125)
                            nc.tensor.matmul(
                                ota[:], vext_s[:, kt, ha * VW:(ha + 1) * VW],
                                e[:, 0, :],
                                start=kt == 0, stop=kt == KT - 1,
                            )
                            nc.tensor.matmul(
                                otb[:], vext_s[:, kt, hb * VW:(hb + 1) * VW],
                                e[:, 1, :],
                                start=kt == 0, stop=kt == KT - 1,
                            )
                            if kt in FILL_SLOTS and pending:
                                pending.popleft()()
                        # evacuate PSUM fast, normalize from SBUF afterwards
                        for nm, ot_ps, psl in (("a", ota, slice(0, 64)),
                                               ("b", otb, slice(64, 128))):
                            otr = att.tile([64, 512], F32, tag="otr", bufs=4,
                                           name=f"otr{nm}_{qb}_{t}")
                            nc.vector.tensor_copy(otr[:], ot_ps[0:64, :])
                            dn = att.tile([1, 512], F32, tag="dn", bufs=4,
                                          name=f"dn{nm}_{qb}_{t}")
                            nc.vector.tensor_copy(dn[:], ot_ps[64:65, :])
                            rd = att.tile([1, 512], F32, tag="rd", bufs=2,
                                          name=f"rd{nm}_{qb}_{t}")
                            nc.vector.reciprocal_approx_fast(rd[:], dn[:])
                            rb = att.tile([64, 512], F32, tag="rb", bufs=2,
                                          name=f"rb{nm}_{qb}_{t}")
                            nc.gpsimd.partition_broadcast(rb[:], rd[:])
                            nc.vector.tensor_tensor(
                                ot_b[psl, t, :], otr[:], rb[:], MULT)

                while pending:
                    pending.popleft()()
                for si in range(4):
                    wo_stage(SB - 1, si, 0)
                    wo_stage(SB - 1, si, 1)
    nc.compile()
    return nc


_NC_CACHE = []


def _tile_x(x):
    # x: [S, D] -> x.T tiled [SB, 128, PKT, 512] with
    # tiled[sb, p, kt, s] = x.T[kt*128 + p, sb*512 + s]
    return np.ascontiguousarray(
        x.T.reshape(PKT, 128, SB, 512).transpose(2, 1, 0, 3))


def _tile_w(wt, nt, m):
    # wt: [D_in, m] (already transposed weight slice) -> [128, nt, m]
    return np.ascontiguousarray(wt.reshape(nt, 128, m).transpose(1, 0, 2))


def kernel(**inputs):
    query = np.asarray(inputs["query"], dtype=np.float32)
    key = np.asarray(inputs["key"], dtype=np.float32)
    value = np.asarray(inputs["value"], dtype=np.float32)
    w_q = np.asarray(inputs["W_q"], dtype=np.float32)
    w_k = np.asarray(inputs["W_k"], dtype=np.float32)
    w_v = np.asarray(inputs["W_v"], dtype=np.float32)
    w_o = np.asarray(inputs["W_o"], dtype=np.float32)

    in_maps = []
    for c in range(8):
        b, hg = c // 2, c % 2
        dsl = slice(hg * DP, (hg + 1) * DP)
        in_maps.append({
            "xqt": _tile_x(query[b]),
            "xkt": _tile_x(key[b]),
            "xvt": _tile_x(value[b]),
            "wqt": _tile_w(w_q[dsl, :].T, PKT, DP),
            "wkt": _tile_w(w_k[dsl, :].T, PKT, DP),
            "wvt": _tile_w(w_v[dsl, :].T, PKT, DP),
            "wot": _tile_w(w_o[:, dsl].T, NT, D),
        })

    if not _NC_CACHE:
        _NC_CACHE.append(build_nc())
    nc = _NC_CACHE[0]
    res = run_bass_kernel_spmd(nc, in_maps, core_ids=list(range(8)),
                               **_RUN_KWARGS)
    _LAST_RESULT.clear()
    _LAST_RESULT.append(res)
    parts = [r["out"] for r in res.results]
    full = np.empty((B, S, D), dtype=np.float32)
    for b in range(B):
        full[b] = parts[2 * b] + parts[2 * b + 1]
    return full


# revision 16
# speedup vs baseline: 1.2223x; 1.0185x over previous
"""MultiHeadAttention Trainium2 Bass kernel, 8-core (batch x head-group) sharded.

Reference computation (B=4, S=2048, D=1024, H=16, d_k=64):
    Q = query @ W_q.T ; K = key @ W_k.T ; V = value @ W_v.T
    per head: attn = softmax(Q K^T / 8) @ V
    out = concat_heads(attn) @ W_o.T

Sharding: core c handles batch b = c // 2 and head-group hg = c % 2 (8 heads,
a 512-wide slice of the model dim). The host pre-transposes and pre-tiles
activations/weights into DMA-friendly layouts (contiguous per SBUF partition);
core-pair partial outputs (row-parallel W_o) are summed on the host while
unsharding.

Per-core dataflow (all matmul inputs float32r; contraction always on the
partition axis):
    K.T[d', s] = (W_k.T slice).T @ x_k.T    (d' on partitions)
    Q.T[d', s] likewise, projected per 512-wide q-block
    V[s, d']   = (x_v.T).T @ W_v.T          (natural layout, + ones column)
    S.T[k, q]  = (K_h.T).T @ Q_h.T          (two heads row-packed, K=64,
                                             both written to one 2-bank tile)
    expS.T     = exp(S.T / 8)               (one ACT op per k-tile, 1024 free)
    O.T+denom  = [V_h | 1].T @ expS.T       (M=65, accumulated over 16 k tiles)
    O.T norm   = O.T * (1/denom)            (copy out of PSUM, then DVE
                                             reciprocal + gpsimd broadcast)
    out[s, :]  = O.T.T @ W_o.T slice        (partial; host adds core pairs)

Scheduling notes: scores->exp->PV runs as a depth-3 pipeline through six PSUM
banks (tag "sc") plus two O.T accumulator banks (tag "ot"); W_o and next-block
Q.T projections are emitted as small chunks inside the k-loop so the in-order
PE stream never starves the scalar engine at block boundaries.
"""
import sys

sys.path.insert(0, "/opt/trn_rl_repo")

import numpy as np

import concourse.bass as bass  # noqa: F401
import concourse.tile as tile
from concourse import bacc, mybir
from concourse.bass_utils import run_bass_kernel_spmd

F32R = mybir.dt.float32r
F32 = mybir.dt.float32
EXP = mybir.ActivationFunctionType.Exp
MULT = mybir.AluOpType.mult

B, S, D = 4, 2048, 1024
H_PER_CORE = 8      # heads per core
DH = 64             # head dim
DP = 512            # per-core model-dim slice (8 heads x 64)
NT = 4              # d' tiles / head pairs per core
SB = 4              # 512-wide s/q blocks
KT = 16             # 128-wide k tiles
PKT = 8             # 128-wide contraction tiles for projections (D / 128)
VW = DH + 1         # V columns per head incl. ones column

_RUN_KWARGS = {}
_LAST_RESULT = []


def build_nc():
    nc = bacc.Bacc("TRN2", target_bir_lowering=False, debug=False)

    # activations pre-tiled on host: [sb, p, kt, 512], contiguous per partition
    xqt = nc.dram_tensor("xqt", [SB, 128, PKT, 512], F32R, kind="ExternalInput")
    xkt = nc.dram_tensor("xkt", [SB, 128, PKT, 512], F32R, kind="ExternalInput")
    xvt = nc.dram_tensor("xvt", [SB, 128, PKT, 512], F32R, kind="ExternalInput")
    # weights pre-tiled: [p, kt, m]
    wqt = nc.dram_tensor("wqt", [128, PKT, DP], F32R, kind="ExternalInput")
    wkt = nc.dram_tensor("wkt", [128, PKT, DP], F32R, kind="ExternalInput")
    wvt = nc.dram_tensor("wvt", [128, PKT, DP], F32R, kind="ExternalInput")
    wot = nc.dram_tensor("wot", [128, NT, D], F32R, kind="ExternalInput")
    out = nc.dram_tensor("out", [S, D], F32, kind="ExternalOutput")

    with tile.TileContext(nc) as tc:
        with tc.tile_pool(name="persist", bufs=1) as persist, \
             tc.tile_pool(name="psum", bufs=3, space="PSUM") as psum:

            # ---- persistent SBUF ----
            wq_s = persist.tile([128, PKT, DP], F32R)
            wot_s = persist.tile([128, NT, D], F32R)
            kt_s = persist.tile([128, NT, S], F32R)          # K.T
            vext_s = persist.tile([128, KT, H_PER_CORE * VW], F32R)  # [V_h | 1]

            # ones columns for the denominator rows (V part is written below)
            ones_f = persist.tile([128, KT, H_PER_CORE], F32)
            nc.vector.memset(ones_f[:], 1.0)
            nc.vector.tensor_copy(
                vext_s[:].rearrange("p k (h c) -> p k h c", c=VW)[:, :, :, DH:DH + 1],
                ones_f[:, :, :, None],
            )

            # ============ phase 1+2: V and K.T projections ============
            with tc.tile_pool(name="proj", bufs=2) as proj:
                wk_s = proj.tile([128, PKT, DP], F32R, bufs=1)
                wv_s = proj.tile([128, PKT, DP], F32R, bufs=1)
                # gpsimd (SWDGE) queue: weights in order of first use
                nc.gpsimd.dma_start(wk_s[:], wkt[:])
                nc.gpsimd.dma_start(wv_s[:], wvt[:])
                nc.gpsimd.dma_start(wq_s[:], wqt[:])

                # K.T and V projections, block-interleaved so each 2MB
                # activation DMA overlaps the previous block's matmuls
                for sb in range(SB):
                    xk_b = proj.tile([128, PKT, 512], F32R, tag="xk",
                                     name=f"xk_{sb}")
                    nc.sync.dma_start(xk_b[:], xkt[sb])
                    for t in range(NT):
                        ps = psum.tile([128, 512], F32, tag="sc",
                                       name=f"psk_{sb}_{t}")
                        for kt in range(PKT):
                            nc.tensor.matmul(
                                ps[:],
                                wk_s[:, kt, t * 128:(t + 1) * 128],
                                xk_b[:, kt, :],
                                start=kt == 0, stop=kt == PKT - 1,
                            )
                        nc.vector.tensor_copy(
                            kt_s[:, t, sb * 512:(sb + 1) * 512], ps[:])
                    xv_b = proj.tile([128, PKT, 512], F32R, tag="xv",
                                     name=f"xv_{sb}")
                    nc.sync.dma_start(xv_b[:], xvt[sb])
                    for half in range(4):
                        st = sb * 4 + half
                        ps = psum.tile([128, 512], F32, tag="sc",
                                       name=f"psv_{st}")
                        for kt in range(PKT):
                            nc.tensor.matmul(
                                ps[:], xv_b[:, kt, half * 128:(half + 1) * 128],
                                wv_s[:, kt, :],
                                start=kt == 0, stop=kt == PKT - 1,
                            )
                        nc.vector.tensor_copy(
                            vext_s[:, st, :].rearrange(
                                "p (h c) -> p h c", c=VW)[:, :, 0:DH],
                            ps[:].rearrange("p (h c) -> p h c", c=DH),
                        )

            # ============ phase 3: attention + W_o, per 512-wide q block ==========
            with tc.tile_pool(name="att", bufs=2) as att:
                qt_tiles = {}

                def xq_load(qb):
                    xq_b = att.tile([128, PKT, 512], F32R, tag="xq", bufs=1,
                                    name=f"xq_{qb}")
                    eng = nc.gpsimd if qb == 0 else nc.sync
                    eng.dma_start(xq_b[:], xqt[qb])
                    qt_tiles[qb] = (att.tile([128, NT, 512], F32R, tag="qt",
                                             name=f"qt_{qb}"), xq_b)

                qt_ps = {}

                def qt_proj(qb, t, half=None):
                    qt_b, xq_b = qt_tiles[qb]
                    if half == 0 or half is None:
                        qt_ps[(qb, t)] = psum.tile([128, 512], F32, tag="sc",
                                                   name=f"psq_{qb}_{t}")
                    ps = qt_ps[(qb, t)]
                    kts = (range(PKT) if half is None else
                           range(half * PKT // 2, (half + 1) * PKT // 2))
                    for kt in kts:
                        nc.tensor.matmul(
                            ps[:], wq_s[:, kt, t * 128:(t + 1) * 128],
                            xq_b[:, kt, :],
                            start=kt == 0, stop=kt == PKT - 1,
                        )
                    if half == 1 or half is None:
                        nc.vector.tensor_copy(qt_b[:, t, :], ps[:])

                def wot_load():
                    nc.gpsimd.dma_start(wot_s[:], wot[:])

                ot_tiles = {}

                def wo_stage(qb, si, dm):
                    ot_b = ot_tiles[qb]
                    st = qb * 4 + si
                    ssl = slice(si * 128, (si + 1) * 128)
                    ps = psum.tile([128, 512], F32, tag="sc",
                                   name=f"pso_{st}_{dm}")
                    for t in range(NT):
                        nc.tensor.matmul(
                            ps[:], ot_b[:, t, ssl],
                            wot_s[:, t, dm * 512:(dm + 1) * 512],
                            start=t == 0, stop=t == NT - 1,
                        )
                    ob = att.tile([128, 512], F32, tag="ob", bufs=3,
                                  name=f"ob_{st}_{dm}")
                    nc.vector.tensor_copy(ob[:], ps[:])
                    nc.sync.dma_start(
                        out[st * 128:(st + 1) * 128, dm * 512:(dm + 1) * 512],
                        ob[:])

                from collections import deque
                from functools import partial

                pending = deque()
                FILL_SLOTS = (2, 5, 8, 11, 14)

                xq_load(0)
                qt_proj(0, 0)
                for tt in range(1, NT):
                    pending.append(partial(qt_proj, 0, tt, 0))
                    pending.append(partial(qt_proj, 0, tt, 1))
                pending.append(wot_load)
                pending.append(partial(xq_load, 1))
                for tt in range(NT):
                    pending.append(partial(qt_proj, 1, tt, 0))
                    pending.append(partial(qt_proj, 1, tt, 1))

                for qb in range(SB):
                    if qb >= 1:
                        # interleave W_o of the previous block with Q.T
                        # projection of the next one
                        nxt = []
                        if qb + 1 < SB:
                            nxt.append(partial(xq_load, qb + 1))
                            for tt in range(NT):
                                nxt.append(partial(qt_proj, qb + 1, tt, 0))
                                nxt.append(partial(qt_proj, qb + 1, tt, 1))
                        wos = [partial(wo_stage, qb - 1, si, dm)
                               for si in range(4) for dm in range(2)]
                        merged = []
                        while nxt or wos:
                            if wos:
                                merged.append(wos.pop(0))
                            if nxt:
                                merged.append(nxt.pop(0))
                        pending.extend(merged)

                    qt_b, _ = qt_tiles[qb]
                    ot_b = att.tile([128, NT, 512], F32R, tag="ot",
                                    name=f"ot_{qb}")
                    ot_tiles[qb] = ot_b
                    for t in range(NT):

                        # two heads: A on partitions 0:64, B on 64:128
                        ota = psum.tile([65, 512], F32, tag="ot", bufs=2,
                                        name=f"ota_{qb}_{t}")
                        otb = psum.tile([65, 512], F32, tag="ot", bufs=2,
                                        name=f"otb_{qb}_{t}")
                        ha, hb = 2 * t, 2 * t + 1
                        for kt in range(KT):
                            ksl = slice(kt * 128, (kt + 1) * 128)
                            sc = psum.tile([128, 2, 512], F32, tag="sc",
                                           name=f"sc_{qb}_{t}_{kt}")
                            nc.tensor.matmul(
                                sc[:, 0, :], kt_s[0:64, t, ksl],
                                qt_b[0:64, t, :],
                                start=True, stop=True, tile_position=(0, 0),
                            )
                            nc.tensor.matmul(
                                sc[:, 1, :], kt_s[64:128, t, ksl],
                                qt_b[64:128, t, :],
                                start=True, stop=True, tile_position=(64, 0),
                            )
                            e = att.tile([128, 2, 512], F32R, tag="exp", bufs=6,
                                         name=f"e_{qb}_{t}_{kt}")
                            nc.scalar.activation(e[:], sc[:], EXP, scale=0.125)
                            nc.tensor.matmul(
                                ota[:], vext_s[:, kt, ha * VW:(ha + 1) * VW],
                                e[:, 0, :],
                                start=kt == 0, stop=kt == KT - 1,
                            )
                            nc.tensor.matmul(
                                otb[:], vext_s[:, kt, hb * VW:(hb + 1) * VW],
                                e[:, 1, :],
                                start=kt == 0, stop=kt == KT - 1,
                            )
                            if kt in FILL_SLOTS and pending:
                                pending.popleft()()
                        # evacuate PSUM fast, normalize from SBUF afterwards
                        for nm, ot_ps, psl in (("a", ota, slice(0, 64)),
                                               ("b", otb, slice(64, 128))):
                            otr = att.tile([64, 512], F32, tag="otr", bufs=4,
                                           name=f"otr{nm}_{qb}_{t}")
                            nc.vector.tensor_copy(otr[:], ot_ps[0:64, :])
                            dn = att.tile([1, 512], F32, tag="dn", bufs=4,
                                          name=f"dn{nm}_{qb}_{t}")
                            nc.vector.tensor_copy(dn[:], ot_ps[64:65, :])
                            rd = att.tile([1, 512], F32, tag="rd", bufs=2,
                                          name=f"rd{nm}_{qb}_{t}")
                            nc.vector.reciprocal_approx_fast(rd[:], dn[:])
                            rb = att.tile([64, 512], F32, tag="rb", bufs=2,
                                          name=f"rb{nm}_{qb}_{t}")
                            nc.gpsimd.partition_broadcast(rb[:], rd[:])
                            nc.vector.tensor_tensor(
                                ot_b[psl, t, :], otr[:], rb[:], MULT)

                while pending:
                    pending.popleft()()
                for si in range(4):
                    wo_stage(SB - 1, si, 0)
                    wo_stage(SB - 1, si, 1)
    nc.compile()
    return nc


_NC_CACHE = []


def _tile_x(x):
    # x: [S, D] -> x.T tiled [SB, 128, PKT, 512] with
    # tiled[sb, p, kt, s] = x.T[kt*128 + p, sb*512 + s]
    return np.ascontiguousarray(
        x.T.reshape(PKT, 128, SB, 512).transpose(2, 1, 0, 3))


def _tile_w(wt, nt, m):
    # wt: [D_in, m] (already transposed weight slice) -> [128, nt, m]
    return np.ascontiguousarray(wt.reshape(nt, 128, m).transpose(1, 0, 2))


def kernel(**inputs):
    query = np.asarray(inputs["query"], dtype=np.float32)
    key = np.asarray(inputs["key"], dtype=np.float32)
    value = np.asarray(inputs["value"], dtype=np.float32)
    w_q = np.asarray(inputs["W_q"], dtype=np.float32)
    w_k = np.asarray(inputs["W_k"], dtype=np.float32)
    w_v = np.asarray(inputs["W_v"], dtype=np.float32)
    w_o = np.asarray(inputs["W_o"], dtype=np.float32)

    in_maps = []
    for c in range(8):
        b, hg = c // 2, c % 2
        dsl = slice(hg * DP, (hg + 1) * DP)
        in_maps.append({
            "xqt": _tile_x(query[b]),
            "xkt": _tile_x(key[b]),
            "xvt": _tile_x(value[b]),
            "wqt": _tile_w(w_q[dsl, :].T, PKT, DP),
            "wkt": _tile_w(w_k[dsl, :].T, PKT, DP),
            "wvt": _tile_w(w_v[dsl, :].T, PKT, DP),
            "wot": _tile_w(w_o[:, dsl].T, NT, D),
        })

    if not _NC_CACHE:
        _NC_CACHE.append(build_nc())
    nc = _NC_CACHE[0]
    res = run_bass_kernel_spmd(nc, in_maps, core_ids=list(range(8)),
                               **_RUN_KWARGS)
    _LAST_RESULT.clear()
    _LAST_RESULT.append(res)
    parts = [r["out"] for r in res.results]
    full = np.empty((B, S, D), dtype=np.float32)
    for b in range(B):
        full[b] = parts[2 * b] + parts[2 * b + 1]
    return full


# revision 20
# speedup vs baseline: 1.2267x; 1.0036x over previous
"""MultiHeadAttention Trainium2 Bass kernel, 8-core (batch x head-group) sharded.

Reference computation (B=4, S=2048, D=1024, H=16, d_k=64):
    Q = query @ W_q.T ; K = key @ W_k.T ; V = value @ W_v.T
    per head: attn = softmax(Q K^T / 8) @ V
    out = concat_heads(attn) @ W_o.T

Sharding: core c handles batch b = c // 2 and head-group hg = c % 2 (8 heads,
a 512-wide slice of the model dim). The host pre-transposes and pre-tiles
activations/weights into DMA-friendly layouts (contiguous per SBUF partition);
core-pair partial outputs (row-parallel W_o) are summed on the host while
unsharding.

Per-core dataflow (all matmul inputs float32r; contraction always on the
partition axis):
    K.T[d', s] = (W_k.T slice).T @ x_k.T    (d' on partitions)
    Q.T[d', s] likewise, projected per 512-wide q-block
    V[s, d']   = (x_v.T).T @ W_v.T          (natural layout, + ones column)
    S.T[k, q]  = (K_h.T).T @ Q_h.T          (two heads row-packed, K=64,
                                             both written to one 2-bank tile)
    expS.T     = exp(S.T / 8)               (one ACT op per k-tile, 1024 free)
    O.T+denom  = [V_h | 1].T @ expS.T       (M=65, accumulated over 16 k tiles)
    O.T norm   = O.T * (1/denom)            (copy out of PSUM, then DVE
                                             reciprocal + gpsimd broadcast)
    out[s, :]  = O.T.T @ W_o.T slice        (partial; host adds core pairs)

Scheduling notes: scores->exp->PV runs as a depth-3 pipeline through six PSUM
banks (tag "sc") plus two O.T accumulator banks (tag "ot"); W_o and next-block
Q.T projections are emitted as small chunks inside the k-loop so the in-order
PE stream never starves the scalar engine at block boundaries.
"""
import sys

sys.path.insert(0, "/opt/trn_rl_repo")

import numpy as np

import concourse.bass as bass  # noqa: F401
import concourse.tile as tile
from concourse import bacc, mybir
from concourse.bass_utils import run_bass_kernel_spmd

F32R = mybir.dt.float32r
F32 = mybir.dt.float32
EXP = mybir.ActivationFunctionType.Exp
MULT = mybir.AluOpType.mult

B, S, D = 4, 2048, 1024
H_PER_CORE = 8      # heads per core
DH = 64             # head dim
DP = 512            # per-core model-dim slice (8 heads x 64)
NT = 4              # d' tiles / head pairs per core
SB = 4              # 512-wide s/q blocks
KT = 16             # 128-wide k tiles
PKT = 8             # 128-wide contraction tiles for projections (D / 128)
VW = DH + 1         # V columns per head incl. ones column

_RUN_KWARGS = {}
_LAST_RESULT = []


def build_nc():
    nc = bacc.Bacc("TRN2", target_bir_lowering=False, debug=False)

    # activations pre-tiled on host: [sb, p, kt, 512], contiguous per partition
    xqt = nc.dram_tensor("xqt", [SB, 128, PKT, 512], F32R, kind="ExternalInput")
    xkt = nc.dram_tensor("xkt", [SB, 128, PKT, 512], F32R, kind="ExternalInput")
    xvt = nc.dram_tensor("xvt", [SB, 128, PKT, 512], F32R, kind="ExternalInput")
    # weights pre-tiled: [p, kt, m]
    wqt = nc.dram_tensor("wqt", [128, PKT, DP], F32R, kind="ExternalInput")
    wkt = nc.dram_tensor("wkt", [128, PKT, DP], F32R, kind="ExternalInput")
    wvt = nc.dram_tensor("wvt", [128, PKT, DP], F32R, kind="ExternalInput")
    wot = nc.dram_tensor("wot", [128, NT, D], F32R, kind="ExternalInput")
    out = nc.dram_tensor("out", [S, D], F32, kind="ExternalOutput")

    with tile.TileContext(nc) as tc:
        with tc.tile_pool(name="persist", bufs=1) as persist, \
             tc.tile_pool(name="psum", bufs=3, space="PSUM") as psum:

            # ---- persistent SBUF ----
            wq_s = persist.tile([128, PKT, DP], F32R)
            wot_s = persist.tile([128, NT, D], F32R)
            kt_s = persist.tile([128, NT, S], F32R)          # K.T
            vext_s = persist.tile([128, KT, H_PER_CORE * VW], F32R)  # [V_h | 1]

            # ones columns for the denominator rows (V part is written below)
            ones_f = persist.tile([128, KT, H_PER_CORE], F32)
            nc.vector.memset(ones_f[:], 1.0)
            nc.vector.tensor_copy(
                vext_s[:].rearrange("p k (h c) -> p k h c", c=VW)[:, :, :, DH:DH + 1],
                ones_f[:, :, :, None],
            )

            # ============ phase 1+2: V and K.T projections ============
            with tc.tile_pool(name="proj", bufs=2) as proj:
                wk_s = proj.tile([128, PKT, DP], F32R, bufs=1)
                wv_s = proj.tile([128, PKT, DP], F32R, bufs=1)
                # gpsimd (SWDGE) queue: weights in order of first use
                nc.gpsimd.dma_start(wk_s[:], wkt[:])
                nc.gpsimd.dma_start(wv_s[:], wvt[:])
                nc.gpsimd.dma_start(wq_s[:], wqt[:])

                # K.T and V projections, block-interleaved so each 2MB
                # activation DMA overlaps the previous block's matmuls
                for sb in range(SB):
                    xk_b = proj.tile([128, PKT, 512], F32R, tag="xk",
                                     name=f"xk_{sb}")
                    nc.sync.dma_start(xk_b[:], xkt[sb])
                    for t in range(NT):
                        ps = psum.tile([128, 512], F32, tag="sc",
                                       name=f"psk_{sb}_{t}")
                        for kt in range(PKT):
                            nc.tensor.matmul(
                                ps[:],
                                wk_s[:, kt, t * 128:(t + 1) * 128],
                                xk_b[:, kt, :],
                                start=kt == 0, stop=kt == PKT - 1,
                            )
                        nc.vector.tensor_copy(
                            kt_s[:, t, sb * 512:(sb + 1) * 512], ps[:])
                    xv_b = proj.tile([128, PKT, 512], F32R, tag="xv",
                                     name=f"xv_{sb}")
                    nc.sync.dma_start(xv_b[:], xvt[sb])
                    for half in range(4):
                        st = sb * 4 + half
                        ps = psum.tile([128, 512], F32, tag="sc",
                                       name=f"psv_{st}")
                        for kt in range(PKT):
                            nc.tensor.matmul(
                                ps[:], xv_b[:, kt, half * 128:(half + 1) * 128],
                                wv_s[:, kt, :],
                                start=kt == 0, stop=kt == PKT - 1,
                            )
                        nc.vector.tensor_copy(
                            vext_s[:, st, :].rearrange(
                                "p (h c) -> p h c", c=VW)[:, :, 0:DH],
                            ps[:].rearrange("p (h c) -> p h c", c=DH),
                        )

            # ============ phase 3: attention + W_o, per 512-wide q block ==========
            with tc.tile_pool(name="att", bufs=2) as att:
                qt_tiles = {}

                def xq_load(qb):
                    xq_b = att.tile([128, PKT, 512], F32R, tag="xq", bufs=1,
                                    name=f"xq_{qb}")
                    eng = nc.gpsimd if qb == 0 else nc.sync
                    eng.dma_start(xq_b[:], xqt[qb])
                    qt_tiles[qb] = (att.tile([128, NT, 512], F32R, tag="qt",
                                             name=f"qt_{qb}"), xq_b)

                qt_ps = {}

                def qt_proj(qb, t, half=None):
                    qt_b, xq_b = qt_tiles[qb]
                    if half == 0 or half is None:
                        qt_ps[(qb, t)] = psum.tile([128, 512], F32, tag="sc",
                                                   name=f"psq_{qb}_{t}")
                    ps = qt_ps[(qb, t)]
                    kts = (range(PKT) if half is None else
                           range(half * PKT // 2, (half + 1) * PKT // 2))
                    for kt in kts:
                        nc.tensor.matmul(
                            ps[:], wq_s[:, kt, t * 128:(t + 1) * 128],
                            xq_b[:, kt, :],
                            start=kt == 0, stop=kt == PKT - 1,
                        )
                    if half == 1 or half is None:
                        nc.vector.tensor_copy(qt_b[:, t, :], ps[:])

                def wot_load():
                    nc.gpsimd.dma_start(wot_s[:], wot[:])

                ot_tiles = {}

                def wo_stage(qb, si, dm):
                    ot_b = ot_tiles[qb]
                    st = qb * 4 + si
                    ssl = slice(si * 128, (si + 1) * 128)
                    ps = psum.tile([128, 512], F32, tag="sc",
                                   name=f"pso_{st}_{dm}")
                    for t in range(NT):
                        nc.tensor.matmul(
                            ps[:], ot_b[:, t, ssl],
                            wot_s[:, t, dm * 512:(dm + 1) * 512],
                            start=t == 0, stop=t == NT - 1,
                        )
                    ob = att.tile([128, 512], F32, tag="ob", bufs=3,
                                  name=f"ob_{st}_{dm}")
                    nc.vector.tensor_copy(ob[:], ps[:])
                    nc.sync.dma_start(
                        out[st * 128:(st + 1) * 128, dm * 512:(dm + 1) * 512],
                        ob[:])

                from collections import deque
                from functools import partial

                pending = deque()
                FILL_SLOTS = (2, 5, 8, 11, 14)

                xq_load(0)
                qt_proj(0, 0)
                for tt in range(1, NT):
                    pending.append(partial(qt_proj, 0, tt, 0))
                    pending.append(partial(qt_proj, 0, tt, 1))
                pending.append(wot_load)
                pending.append(partial(xq_load, 1))
                for tt in range(NT):
                    pending.append(partial(qt_proj, 1, tt, 0))
                    pending.append(partial(qt_proj, 1, tt, 1))

                for qb in range(SB):
                    if qb >= 1:
                        # interleave W_o of the previous block with Q.T
                        # projection of the next one
                        nxt = []
                        if qb + 1 < SB:
                            nxt.append(partial(xq_load, qb + 1))
                            for tt in range(NT):
                                nxt.append(partial(qt_proj, qb + 1, tt, 0))
                                nxt.append(partial(qt_proj, qb + 1, tt, 1))
                        wos = [partial(wo_stage, qb - 1, si, dm)
                               for si in range(4) for dm in range(2)]
                        merged = []
                        while nxt or wos:
                            if wos:
                                merged.append(wos.pop(0))
                            if nxt:
                                merged.append(nxt.pop(0))
                        pending.extend(merged)

                    qt_b, _ = qt_tiles[qb]
                    ot_b = att.tile([128, NT, 512], F32R, tag="ot",
                                    name=f"ot_{qb}")
                    ot_tiles[qb] = ot_b
                    for t in range(NT):

                        # two heads: A on partitions 0:64, B on 64:128
                        ota = psum.tile([65, 512], F32, tag="ot", bufs=2,
                                        name=f"ota_{qb}_{t}")
                        otb = psum.tile([65, 512], F32, tag="ot", bufs=2,
                                        name=f"otb_{qb}_{t}")
                        ha, hb = 2 * t, 2 * t + 1
                        for kt in range(KT):
                            ksl = slice(kt * 128, (kt + 1) * 128)
                            sc = psum.tile([128, 2, 512], F32, tag="sc",
                                           name=f"sc_{qb}_{t}_{kt}")
                            nc.tensor.matmul(
                                sc[:, 0, :], kt_s[0:64, t, ksl],
                                qt_b[0:64, t, :],
                                start=True, stop=True, tile_position=(0, 0),
                            )
                            nc.tensor.matmul(
                                sc[:, 1, :], kt_s[64:128, t, ksl],
                                qt_b[64:128, t, :],
                                start=True, stop=True, tile_position=(64, 0),
                            )
                            e = att.tile([128, 2, 512], F32R, tag="exp", bufs=6,
                                         name=f"e_{qb}_{t}_{kt}")
                            nc.scalar.activation(e[:], sc[:], EXP, scale=0.125)
                            nc.tensor.matmul(
                                ota[:], vext_s[:, kt, ha * VW:(ha + 1) * VW],
                                e[:, 0, :],
                                start=kt == 0, stop=kt == KT - 1,
                            )
                            nc.tensor.matmul(
                                otb[:], vext_s[:, kt, hb * VW:(hb + 1) * VW],
                                e[:, 1, :],
                                start=kt == 0, stop=kt == KT - 1,
                            )
                            if kt in FILL_SLOTS and pending:
                                pending.popleft()()
                        # evacuate PSUM fast, normalize from SBUF afterwards
                        for nm, ot_ps, psl in (("a", ota, slice(0, 64)),
                                               ("b", otb, slice(64, 128))):
                            otr = att.tile([64, 512], F32, tag="otr", bufs=4,
                                           name=f"otr{nm}_{qb}_{t}")
                            nc.vector.tensor_copy(otr[:], ot_ps[0:64, :])
                            dn = att.tile([1, 512], F32, tag="dn", bufs=4,
                                          name=f"dn{nm}_{qb}_{t}")
                            nc.vector.tensor_copy(dn[:], ot_ps[64:65, :])
                            rd = att.tile([1, 512], F32, tag="rd", bufs=2,
                                          name=f"rd{nm}_{qb}_{t}")
                            nc.vector.reciprocal_approx_fast(rd[:], dn[:])
                            rb = att.tile([64, 512], F32, tag="rb", bufs=2,
                                          name=f"rb{nm}_{qb}_{t}")
                            nc.gpsimd.partition_broadcast(rb[:], rd[:])
                            nc.vector.tensor_tensor(
                                ot_b[psl, t, :], otr[:], rb[:], MULT)

                while pending:
                    pending.popleft()()
                for si in range(4):
                    wo_stage(SB - 1, si, 0)
                    wo_stage(SB - 1, si, 1)
    nc.compile()
    return nc


_NC_CACHE = []


def _tile_x(x):
    # x: [S, D] -> x.T tiled [SB, 128, PKT, 512] with
    # tiled[sb, p, kt, s] = x.T[kt*128 + p, sb*512 + s]
    return np.ascontiguousarray(
        x.T.reshape(PKT, 128, SB, 512).transpose(2, 1, 0, 3))


def _tile_w(wt, nt, m):
    # wt: [D_in, m] (already transposed weight slice) -> [128, nt, m]
    return np.ascontiguousarray(wt.reshape(nt, 128, m).transpose(1, 0, 2))


def kernel(**inputs):
    query = np.asarray(inputs["query"], dtype=np.float32)
    key = np.asarray(inputs["key"], dtype=np.float32)
    value = np.asarray(inputs["value"], dtype=np.float32)
    w_q = np.asarray(inputs["W_q"], dtype=np.float32)
    w_k = np.asarray(inputs["W_k"], dtype=np.float32)
    w_v = np.asarray(inputs["W_v"], dtype=np.float32)
    w_o = np.asarray(inputs["W_o"], dtype=np.float32)

    in_maps = []
    for c in range(8):
        b, hg = c // 2, c % 2
        dsl = slice(hg * DP, (hg + 1) * DP)
        in_maps.append({
            "xqt": _tile_x(query[b]),
            "xkt": _tile_x(key[b]),
            "xvt": _tile_x(value[b]),
            "wqt": _tile_w(w_q[dsl, :].T, PKT, DP),
            "wkt": _tile_w(w_k[dsl, :].T, PKT, DP),
            "wvt": _tile_w(w_v[dsl, :].T, PKT, DP),
            "wot": _tile_w(w_o[:, dsl].T, NT, D),
        })

    if not _NC_CACHE:
        _NC_CACHE.append(build_nc())
    nc = _NC_CACHE[0]
    res = run_bass_kernel_spmd(nc, in_maps, core_ids=list(range(8)),
                               **_RUN_KWARGS)
    _LAST_RESULT.clear()
    _LAST_RESULT.append(res)
    parts = [r["out"] for r in res.results]
    full = np.empty((B, S, D), dtype=np.float32)
    for b in range(B):
        full[b] = parts[2 * b] + parts[2 * b + 1]
    return full
